# revision 7
# baseline (speedup 1.0000x reference)
"""Conv1D (B=32, L=8192, C_in=64, K=3, F=128, VALID) + bias + ReLU on 8 trn2 cores.

Strategy: data-parallel over batch (4 batches per core). Per core:
  - DMA x[b] in position-chunks, p-major layout ([128, T*64] tiles, 2KB/partition
    contiguous per partition).
  - PE-transpose [128,128] sub-tiles to build x_T [C=64, L] in SBUF (plus a
    row of ones at partition 64 so the bias can ride along a K=65 matmul).
  - out[pos, F] tile = sum of 3 accumulated matmuls (k=0 carries bias row),
    contraction over C on partitions.
  - ReLU fused on ScalarE (PSUM -> SBUF), staged into big tiles, contiguous
    DMA out.
"""

import os
import sys

import numpy as np

_TRN_REPO = "/opt/trn_rl_repo"
if _TRN_REPO not in sys.path and os.path.isdir(_TRN_REPO):
    sys.path.insert(0, _TRN_REPO)

import concourse.bass as bass
import concourse.tile as tile
from concourse import bacc, mybir
from concourse.bass_utils import run_bass_kernel_spmd
from concourse.masks import make_identity

B, L, C = 32, 8192, 64
K, F = 3, 128
L_OUT = L - K + 1  # 8190
N_CORES = 8
B_SHARD = B // N_CORES  # 4

# Matmul operand dtype: float32 (exact) or bfloat16 (fast).
MM_DT = mybir.dt.float32

IN_CHUNK = 2048  # positions per input DMA chunk (16 x 128)
T_IN = IN_CHUNK // 128  # 16
OUT_CHUNK = 2048  # positions per output staging tile


def _conv_kernel(tc: tile.TileContext, out_ap, x_ap, w_ap, b_ap, mm_dt):
    nc = tc.nc
    fp32 = mybir.dt.float32

    with (
        tc.tile_pool(name="setup", bufs=1) as setup_pool,
        tc.tile_pool(name="xin", bufs=3) as xin_pool,
        tc.tile_pool(name="xT", bufs=2) as xT_pool,
        tc.tile_pool(name="osb", bufs=2) as osb_pool,
        tc.tile_pool(name="pt", bufs=4, space="PSUM") as pt_pool,
        tc.tile_pool(name="po", bufs=4, space="PSUM") as po_pool,
    ):
        # --- one-time setup: weights, bias, identity ---
        wstage = setup_pool.tile([C, K * F], fp32)
        for k in range(K):
            nc.sync.dma_start(out=wstage[:, k * F : (k + 1) * F], in_=w_ap[k])
        bstage = setup_pool.tile([1, F], fp32)
        nc.sync.dma_start(out=bstage[:, :], in_=b_ap[None, :])

        # w0b carries the bias as row 64 (pairs with the ones-row of x_T).
        w0b = setup_pool.tile([C + 1, F], mm_dt)
        nc.vector.tensor_copy(w0b[0:C, :], wstage[:, 0:F])
        nc.vector.tensor_copy(w0b[C : C + 1, :], bstage[:, :])
        w1 = setup_pool.tile([C, F], mm_dt)
        nc.vector.tensor_copy(w1[:, :], wstage[:, F : 2 * F])
        w2 = setup_pool.tile([C, F], mm_dt)
        nc.vector.tensor_copy(w2[:, :], wstage[:, 2 * F : 3 * F])

        ident = setup_pool.tile([128, 128], fp32)
        make_identity(nc, ident)

        for b in range(B_SHARD):
            # --- build x_T [C+1, L] for this batch ---
            xT = xT_pool.tile([C + 1, L], mm_dt)
            nc.gpsimd.memset(xT[C : C + 1, :], 1.0)
            # view with columns split as (n, s): col = 16*n + s
            xT_r = xT.rearrange("c (n s) -> c n s", s=T_IN)

            for ci in range(L // IN_CHUNK):
                c0 = ci * IN_CHUNK
                xin = xin_pool.tile([128, T_IN * C], fp32)
                nc.sync.dma_start(
                    out=xin.rearrange("p (t c) -> p t c", c=C),
                    in_=x_ap[b, c0 : c0 + IN_CHUNK, :].rearrange(
                        "(p t) c -> p t c", p=128
                    ),
                )
                # transpose [128,128] sub-tiles; each holds t in {2j, 2j+1}
                for j in range(T_IN * C // 128):
                    pt = pt_pool.tile([128, 128], fp32)
                    nc.tensor.transpose(pt[:, :], xin[:, j * 128 : (j + 1) * 128], ident)
                    for tt in range(2):
                        s = 2 * j + tt
                        nc.vector.tensor_copy(
                            xT_r[0:C, c0 // T_IN : c0 // T_IN + 128, s],
                            pt[tt * C : (tt + 1) * C, :],
                        )

            # --- matmuls + relu + store ---
            for oc in range((L_OUT + OUT_CHUNK - 1) // OUT_CHUNK):
                o0 = oc * OUT_CHUNK
                opos = min(OUT_CHUNK, L_OUT - o0)  # 2048 or 2046
                n_full = opos // 128
                tail = opos - n_full * 128
                osb = osb_pool.tile([128, OUT_CHUNK], fp32)
                for t in range(n_full + (1 if tail else 0)):
                    p0 = o0 + t * 128
                    P = min(128, L_OUT - p0)
                    po = po_pool.tile([128, F], fp32)
                    nc.tensor.matmul(
                        po[0:P, :],
                        xT[0 : C + 1, p0 : p0 + P],
                        w0b[:, :],
                        start=True,
                        stop=False,
                    )
                    nc.tensor.matmul(
                        po[0:P, :],
                        xT[0:C, p0 + 1 : p0 + 1 + P],
                        w1[:, :],
                        start=False,
                        stop=False,
                    )
                    nc.tensor.matmul(
                        po[0:P, :],
                        xT[0:C, p0 + 2 : p0 + 2 + P],
                        w2[:, :],
                        start=False,
                        stop=True,
                    )
                    nc.scalar.activation(
                        osb[0:P, t * F : (t + 1) * F],
                        po[0:P, :],
                        mybir.ActivationFunctionType.Relu,
                    )
                # store: full tiles in one big DMA, tail tile separately
                if n_full:
                    nc.sync.dma_start(
                        out=out_ap[b, o0 : o0 + n_full * 128, :].rearrange(
                            "(t p) f -> p t f", p=128
                        ),
                        in_=osb[:, 0 : n_full * F].rearrange("p (t f) -> p t f", f=F),
                    )
                if tail:
                    nc.sync.dma_start(
                        out=out_ap[b, o0 + n_full * 128 : o0 + opos, :],
                        in_=osb[0:tail, n_full * F : (n_full + 1) * F],
                    )


def build_program(mm_dt=MM_DT):
    nc = bacc.Bacc("TRN2", target_bir_lowering=False, debug=False)
    x = nc.dram_tensor("x", [B_SHARD, L, C], mybir.dt.float32, kind="ExternalInput")
    w = nc.dram_tensor("w", [K, C, F], mybir.dt.float32, kind="ExternalInput")
    bb = nc.dram_tensor("b", [F], mybir.dt.float32, kind="ExternalInput")
    out = nc.dram_tensor(
        "out", [B_SHARD, L_OUT, F], mybir.dt.float32, kind="ExternalOutput"
    )
    with tile.TileContext(nc) as tc:
        _conv_kernel(tc, out.ap(), x.ap(), w.ap(), bb.ap(), mm_dt)
    nc.compile()
    return nc


def kernel(x, w, b, _trace=False, _trace_kwargs=None):
    x = np.ascontiguousarray(np.asarray(x, dtype=np.float32))
    w = np.ascontiguousarray(np.asarray(w, dtype=np.float32))
    b = np.ascontiguousarray(np.asarray(b, dtype=np.float32))
    assert x.shape == (B, L, C) and w.shape == (K, C, F) and b.shape == (F,)

    nc = build_program()
    in_maps = [
        {"x": x[i * B_SHARD : (i + 1) * B_SHARD], "w": w, "b": b}
        for i in range(N_CORES)
    ]
    res = run_bass_kernel_spmd(
        nc,
        in_maps,
        core_ids=list(range(N_CORES)),
        trace=_trace,
        **(_trace_kwargs or {}),
    )
    out = np.concatenate([r["out"] for r in res.results], axis=0)
    if _trace:
        return out, res
    return out


if __name__ == "__main__":
    rng = np.random.default_rng(0)
    x = rng.standard_normal((B, L, C), dtype=np.float32)
    w = rng.standard_normal((K, C, F), dtype=np.float32) * 0.08
    b = np.zeros((F,), dtype=np.float32)
    out = kernel(x, w, b)
    print("out", out.shape, out.dtype, float(np.abs(out).max()))


# revision 8
# speedup vs baseline: 2.5141x; 2.5141x over previous
"""Conv1D (B=32, L=8192, C_in=64, K=3, F=128, VALID) + bias + ReLU on 8 trn2 cores.

Strategy: data-parallel over batch (4 batches per core). Per core:
  - DMA x[b] in position-chunks, p-major layout ([128, T*64] tiles, 2KB/partition
    contiguous per partition).
  - PE-transpose [128,128] sub-tiles to build x_T [C=64, L] in SBUF (plus a
    row of ones at partition 64 so the bias can ride along a K=65 matmul).
  - out[pos, F] tile = sum of 3 accumulated matmuls (k=0 carries bias row),
    contraction over C on partitions.
  - ReLU fused on ScalarE (PSUM -> SBUF), staged into big tiles, contiguous
    DMA out.
"""

import os
import sys

import numpy as np

_TRN_REPO = "/opt/trn_rl_repo"
if _TRN_REPO not in sys.path and os.path.isdir(_TRN_REPO):
    sys.path.insert(0, _TRN_REPO)

import concourse.bass as bass
import concourse.tile as tile
from concourse import bacc, mybir
from concourse.bass_utils import run_bass_kernel_spmd
from concourse.masks import make_identity

B, L, C = 32, 8192, 64
K, F = 3, 128
L_OUT = L - K + 1  # 8190
N_CORES = 8
B_SHARD = B // N_CORES  # 4

# Matmul operand dtype: float32 (exact) or bfloat16 (fast).
MM_DT = mybir.dt.bfloat16

IN_CHUNK = 2048  # positions per input DMA chunk (16 x 128)
T_IN = IN_CHUNK // 128  # 16
OUT_CHUNK = 2048  # positions per output staging tile


def _conv_kernel(tc: tile.TileContext, out_ap, x_ap, w_ap, b_ap, mm_dt):
    nc = tc.nc
    fp32 = mybir.dt.float32

    with (
        tc.tile_pool(name="setup", bufs=1) as setup_pool,
        tc.tile_pool(name="xin", bufs=3) as xin_pool,
        tc.tile_pool(name="xT", bufs=2) as xT_pool,
        tc.tile_pool(name="osb", bufs=2) as osb_pool,
        tc.tile_pool(name="pt", bufs=4, space="PSUM") as pt_pool,
        tc.tile_pool(name="po", bufs=4, space="PSUM") as po_pool,
    ):
        # --- one-time setup: weights, bias, identity ---
        wstage = setup_pool.tile([C, K * F], fp32)
        for k in range(K):
            nc.sync.dma_start(out=wstage[:, k * F : (k + 1) * F], in_=w_ap[k])
        bstage = setup_pool.tile([1, F], fp32)
        nc.sync.dma_start(out=bstage[:, :], in_=b_ap[None, :])

        # w0b carries the bias as row 64 (pairs with the ones-row of x_T).
        w0b = setup_pool.tile([C + 1, F], mm_dt)
        nc.vector.tensor_copy(w0b[0:C, :], wstage[:, 0:F])
        nc.vector.tensor_copy(w0b[C : C + 1, :], bstage[:, :])
        w1 = setup_pool.tile([C, F], mm_dt)
        nc.vector.tensor_copy(w1[:, :], wstage[:, F : 2 * F])
        w2 = setup_pool.tile([C, F], mm_dt)
        nc.vector.tensor_copy(w2[:, :], wstage[:, 2 * F : 3 * F])

        ident = setup_pool.tile([128, 128], fp32)
        make_identity(nc, ident)

        for b in range(B_SHARD):
            # --- build x_T [C+1, L] for this batch ---
            xT = xT_pool.tile([C + 1, L], mm_dt)
            nc.gpsimd.memset(xT[C : C + 1, :], 1.0)
            # view with columns split as (n, s): col = 16*n + s
            xT_r = xT.rearrange("c (n s) -> c n s", s=T_IN)

            for ci in range(L // IN_CHUNK):
                c0 = ci * IN_CHUNK
                xin = xin_pool.tile([128, T_IN * C], fp32)
                nc.sync.dma_start(
                    out=xin.rearrange("p (t c) -> p t c", c=C),
                    in_=x_ap[b, c0 : c0 + IN_CHUNK, :].rearrange(
                        "(p t) c -> p t c", p=128
                    ),
                )
                # transpose [128,128] sub-tiles; each holds t in {2j, 2j+1}
                for j in range(T_IN * C // 128):
                    pt = pt_pool.tile([128, 128], fp32)
                    nc.tensor.transpose(pt[:, :], xin[:, j * 128 : (j + 1) * 128], ident)
                    for tt in range(2):
                        s = 2 * j + tt
                        nc.vector.tensor_copy(
                            xT_r[0:C, c0 // T_IN : c0 // T_IN + 128, s],
                            pt[tt * C : (tt + 1) * C, :],
                        )

            # --- matmuls + relu + store ---
            for oc in range((L_OUT + OUT_CHUNK - 1) // OUT_CHUNK):
                o0 = oc * OUT_CHUNK
                opos = min(OUT_CHUNK, L_OUT - o0)  # 2048 or 2046
                n_full = opos // 128
                tail = opos - n_full * 128
                osb = osb_pool.tile([128, OUT_CHUNK], fp32)
                for t in range(n_full + (1 if tail else 0)):
                    p0 = o0 + t * 128
                    P = min(128, L_OUT - p0)
                    po = po_pool.tile([128, F], fp32)
                    nc.tensor.matmul(
                        po[0:P, :],
                        xT[0 : C + 1, p0 : p0 + P],
                        w0b[:, :],
                        start=True,
                        stop=False,
                    )
                    nc.tensor.matmul(
                        po[0:P, :],
                        xT[0:C, p0 + 1 : p0 + 1 + P],
                        w1[:, :],
                        start=False,
                        stop=False,
                    )
                    nc.tensor.matmul(
                        po[0:P, :],
                        xT[0:C, p0 + 2 : p0 + 2 + P],
                        w2[:, :],
                        start=False,
                        stop=True,
                    )
                    nc.scalar.activation(
                        osb[0:P, t * F : (t + 1) * F],
                        po[0:P, :],
                        mybir.ActivationFunctionType.Relu,
                    )
                # store: full tiles in one big DMA, tail tile separately
                if n_full:
                    nc.sync.dma_start(
                        out=out_ap[b, o0 : o0 + n_full * 128, :].rearrange(
                            "(t p) f -> p t f", p=128
                        ),
                        in_=osb[:, 0 : n_full * F].rearrange("p (t f) -> p t f", f=F),
                    )
                if tail:
                    nc.sync.dma_start(
                        out=out_ap[b, o0 + n_full * 128 : o0 + opos, :],
                        in_=osb[0:tail, n_full * F : (n_full + 1) * F],
                    )


def build_program(mm_dt=MM_DT):
    nc = bacc.Bacc("TRN2", target_bir_lowering=False, debug=False)
    x = nc.dram_tensor("x", [B_SHARD, L, C], mybir.dt.float32, kind="ExternalInput")
    w = nc.dram_tensor("w", [K, C, F], mybir.dt.float32, kind="ExternalInput")
    bb = nc.dram_tensor("b", [F], mybir.dt.float32, kind="ExternalInput")
    out = nc.dram_tensor(
        "out", [B_SHARD, L_OUT, F], mybir.dt.float32, kind="ExternalOutput"
    )
    with tile.TileContext(nc) as tc:
        _conv_kernel(tc, out.ap(), x.ap(), w.ap(), bb.ap(), mm_dt)
    nc.compile()
    return nc


def kernel(x, w, b, _trace=False, _trace_kwargs=None):
    x = np.ascontiguousarray(np.asarray(x, dtype=np.float32))
    w = np.ascontiguousarray(np.asarray(w, dtype=np.float32))
    b = np.ascontiguousarray(np.asarray(b, dtype=np.float32))
    assert x.shape == (B, L, C) and w.shape == (K, C, F) and b.shape == (F,)

    nc = build_program()
    in_maps = [
        {"x": x[i * B_SHARD : (i + 1) * B_SHARD], "w": w, "b": b}
        for i in range(N_CORES)
    ]
    res = run_bass_kernel_spmd(
        nc,
        in_maps,
        core_ids=list(range(N_CORES)),
        trace=_trace,
        **(_trace_kwargs or {}),
    )
    out = np.concatenate([r["out"] for r in res.results], axis=0)
    if _trace:
        return out, res
    return out


if __name__ == "__main__":
    rng = np.random.default_rng(0)
    x = rng.standard_normal((B, L, C), dtype=np.float32)
    w = rng.standard_normal((K, C, F), dtype=np.float32) * 0.08
    b = np.zeros((F,), dtype=np.float32)
    out = kernel(x, w, b)
    print("out", out.shape, out.dtype, float(np.abs(out).max()))


# revision 12
# speedup vs baseline: 3.8069x; 1.5142x over previous
"""Conv1D (B=32, L=8192, C_in=64, K=3, F=128, VALID) + bias + ReLU on 8 trn2 cores.

Strategy: data-parallel over batch (4 batches per core). Per core:
  - DMA x[b] in position-chunks (t-major [128, T*64] tiles).
  - Cast fp32 -> bf16 on GpSimd, PE-transpose [128,128] bf16 sub-tiles,
    contiguous-copy PSUM halves into x_T [128, L] in SBUF:
    rows 0-63 channels, row 64 ones (bias rides a K=128 matmul row),
    rows 65-127 zero (pad so every matmul is K=128 -> fast weight load).
  - out[pos, F]: 3 accumulated K=128 matmuls per 128-position tile into a
    [128, 512] PSUM bank; one ReLU (ScalarE) per bank -> SBUF; contiguous
    DMA out.
"""

import os
import sys

import numpy as np

_TRN_REPO = "/opt/trn_rl_repo"
if _TRN_REPO not in sys.path and os.path.isdir(_TRN_REPO):
    sys.path.insert(0, _TRN_REPO)

import concourse.bass as bass
import concourse.tile as tile
from concourse import bacc, mybir
from concourse.bass_utils import run_bass_kernel_spmd
from concourse.masks import make_identity

B, L, C = 32, 8192, 64
K, F = 3, 128
L_OUT = L - K + 1  # 8190
N_CORES = 8
B_SHARD = B // N_CORES  # 4

MM_DT = mybir.dt.bfloat16

IN_CHUNK = 2048  # positions per input DMA chunk (16 x 128)
T_IN = IN_CHUNK // 128  # 16
PO_CHUNK = 512  # positions per PSUM output bank
OUT_CHUNK = 2048  # positions per output staging tile


def _conv_kernel(tc: tile.TileContext, out_ap, x_ap, w_ap, b_ap, mm_dt):
    nc = tc.nc
    fp32 = mybir.dt.float32

    with (
        tc.tile_pool(name="setup", bufs=1) as setup_pool,
        tc.tile_pool(name="xin", bufs=3) as xin_pool,
        tc.tile_pool(name="xbf", bufs=3) as xbf_pool,
        tc.tile_pool(name="osb", bufs=2) as osb_pool,
        tc.tile_pool(name="pt", bufs=4, space="PSUM") as pt_pool,
        tc.tile_pool(name="po", bufs=3, space="PSUM") as po_pool,
    ):
        # --- one-time setup: weights, bias, identity, xT double-buffer ---
        wstage = setup_pool.tile([C, K * F], fp32)
        for k in range(K):
            nc.sync.dma_start(out=wstage[:, k * F : (k + 1) * F], in_=w_ap[k])
        bstage = setup_pool.tile([1, F], fp32)
        nc.sync.dma_start(out=bstage[:, :], in_=b_ap[None, :])

        # padded weights: w0b rows 0-63 = w[0], row 64 = bias, rest zero;
        # w1p/w2p rows 0-63 = w[k], rest zero.
        wpad = setup_pool.tile([128, K * F], mm_dt)
        nc.vector.memset(wpad[:, :], 0.0)
        for k in range(K):
            nc.vector.tensor_copy(wpad[0:C, k * F : (k + 1) * F], wstage[:, k * F : (k + 1) * F])
        nc.vector.tensor_copy(wpad[C : C + 1, 0:F], bstage[:, :])

        ident = setup_pool.tile([128, 128], mm_dt)
        make_identity(nc, ident)

        # xT: manually double-buffered [128, 2*L]; row 64 ones / rows 65-127
        # zero are set once and never rewritten.
        xT = setup_pool.tile([128, 2 * L], mm_dt)
        nc.vector.memset(xT[C:128, :], 0.0)
        nc.vector.memset(xT[C : C + 1, :], 1.0)

        for b in range(B_SHARD):
            xTb = xT[:, (b % 2) * L : (b % 2) * L + L]
            for ci in range(L // IN_CHUNK):
                c0 = ci * IN_CHUNK
                xin = xin_pool.tile([128, T_IN * C], fp32)
                nc.sync.dma_start(
                    out=xin.rearrange("p (t c) -> p t c", c=C),
                    in_=x_ap[b, c0 : c0 + IN_CHUNK, :].rearrange(
                        "(t p) c -> p t c", p=128
                    ),
                )
                xbf = xbf_pool.tile([128, T_IN * C], mm_dt)
                nc.gpsimd.tensor_copy(xbf[:, :], xin[:, :])
                # transpose [128,128] sub-tiles; sub-tile j holds t in {2j, 2j+1}
                for j in range(T_IN * C // 128):
                    pt = pt_pool.tile([128, 128], mm_dt)
                    nc.tensor.transpose(pt[:, :], xbf[:, j * 128 : (j + 1) * 128], ident)
                    for tt in range(2):
                        q = c0 + (2 * j + tt) * 128
                        nc.vector.tensor_copy(
                            xTb[0:C, q : q + 128],
                            pt[tt * C : (tt + 1) * C, :],
                        )

            # --- matmuls + relu + store ---
            for oc in range((L_OUT + OUT_CHUNK - 1) // OUT_CHUNK):
                o0 = oc * OUT_CHUNK
                opos = min(OUT_CHUNK, L_OUT - o0)  # 2048 or 2046
                osb = osb_pool.tile([128, OUT_CHUNK], fp32)
                for pc in range((opos + PO_CHUNK - 1) // PO_CHUNK):
                    g0 = o0 + pc * PO_CHUNK
                    gpos = min(PO_CHUNK, L_OUT - g0)  # 512 or 510
                    po = po_pool.tile([128, PO_CHUNK], fp32)
                    n_sub = (gpos + 127) // 128
                    for t in range(n_sub):
                        p0 = g0 + t * 128
                        P = min(128, L_OUT - p0)
                        sub = po[0:P, t * F : (t + 1) * F]
                        for k in range(K):
                            nc.tensor.matmul(
                                sub,
                                xTb[:, p0 + k : p0 + k + P],
                                wpad[:, k * F : (k + 1) * F],
                                start=(k == 0),
                                stop=(k == K - 1),
                            )
                    full_sub = gpos // 128
                    tail_sub = gpos - full_sub * 128
                    ob = pc * PO_CHUNK
                    if full_sub:
                        nc.scalar.activation(
                            osb[:, ob : ob + full_sub * F],
                            po[:, 0 : full_sub * F],
                            mybir.ActivationFunctionType.Relu,
                        )
                    if tail_sub:
                        nc.scalar.activation(
                            osb[0:tail_sub, ob + full_sub * F : ob + n_sub * F],
                            po[0:tail_sub, full_sub * F : n_sub * F],
                            mybir.ActivationFunctionType.Relu,
                        )
                # store: full tiles in one big DMA, tail tile separately
                n_full = opos // 128
                tail = opos - n_full * 128
                if n_full:
                    nc.sync.dma_start(
                        out=out_ap[b, o0 : o0 + n_full * 128, :].rearrange(
                            "(t p) f -> p t f", p=128
                        ),
                        in_=osb[:, 0 : n_full * F].rearrange("p (t f) -> p t f", f=F),
                    )
                if tail:
                    nc.sync.dma_start(
                        out=out_ap[b, o0 + n_full * 128 : o0 + opos, :],
                        in_=osb[0:tail, n_full * F : (n_full + 1) * F],
                    )


def build_program(mm_dt=MM_DT):
    nc = bacc.Bacc("TRN2", target_bir_lowering=False, debug=False)
    x = nc.dram_tensor("x", [B_SHARD, L, C], mybir.dt.float32, kind="ExternalInput")
    w = nc.dram_tensor("w", [K, C, F], mybir.dt.float32, kind="ExternalInput")
    bb = nc.dram_tensor("b", [F], mybir.dt.float32, kind="ExternalInput")
    out = nc.dram_tensor(
        "out", [B_SHARD, L_OUT, F], mybir.dt.float32, kind="ExternalOutput"
    )
    with tile.TileContext(nc) as tc:
        _conv_kernel(tc, out.ap(), x.ap(), w.ap(), bb.ap(), mm_dt)
    nc.compile()
    return nc


def kernel(x, w, b, _trace=False, _trace_kwargs=None):
    x = np.ascontiguousarray(np.asarray(x, dtype=np.float32))
    w = np.ascontiguousarray(np.asarray(w, dtype=np.float32))
    b = np.ascontiguousarray(np.asarray(b, dtype=np.float32))
    assert x.shape == (B, L, C) and w.shape == (K, C, F) and b.shape == (F,)

    nc = build_program()
    in_maps = [
        {"x": x[i * B_SHARD : (i + 1) * B_SHARD], "w": w, "b": b}
        for i in range(N_CORES)
    ]
    res = run_bass_kernel_spmd(
        nc,
        in_maps,
        core_ids=list(range(N_CORES)),
        trace=_trace,
        **(_trace_kwargs or {}),
    )
    out = np.concatenate([r["out"] for r in res.results], axis=0)
    if _trace:
        return out, res
    return out


if __name__ == "__main__":
    rng = np.random.default_rng(0)
    x = rng.standard_normal((B, L, C), dtype=np.float32)
    w = rng.standard_normal((K, C, F), dtype=np.float32) * 0.08
    b = np.zeros((F,), dtype=np.float32)
    out = kernel(x, w, b)
    print("out", out.shape, out.dtype, float(np.abs(out).max()))
